# revision 28
# baseline (speedup 1.0000x reference)
"""Trainium2 Bass kernel for nn_Attention_aggregator (B=8, N=4096, F=128, E=128).

Sharding: data-parallel over batch — one batch element per NeuronCore (8 cores).
Each core computes, for its batch b:
    att  = x @ x.T                        [N, N]   (symmetric)
    att  = where(adj==0, -9999999, att)
    sm   = softmax(att, axis=-1)
    comb = sm @ x                         [N, F]
    out  = relu(concat([x, comb], -1) @ W.T)      [N, E]

Device decomposition (transposed orientation so the aggregation matmul's
contraction dim lands on partitions; attention symmetry makes the transposed
logits free):
    E^T[m, r] = exp(att[m, r] - 80)       (att[m,r] == att[r,m])
    diagonal of att killed in PSUM by an accumulating (-30000*I) @ I matmul
    P^T = E^T * adjT                      (adjT prepacked to bf16 on host)
    [S2 | S1] = P^T.T @ [x | 1]           (ones column => row-sum in column F)
    comb = (ev*S2 + coef*x) / (ev*S1 + coef)   with d = ||x_r||^2,
        ev = exp(-adj_rr*max(0, d-110)),  coef = adj_rr*exp(min(d-80, 30))
    (diagonal handled analytically: its logit is the only one that can
     overflow exp; everything off-diagonal is bounded ~|att|<70)
    out = relu([x, comb] @ W.T) with comb'^T stationary matmuls.

Perf structure vs the previous version:
  - exp is issued as one ACTIVATE per TWO 512-wide tiles ([128,1024] PSUM
    region, 2 banks) — halves the ~300ns fixed ACT overhead per tile.
  - adjacency is host-converted to bf16 (values 0/1 exact) and streamed
    contiguously — 2x DMA efficiency vs the strided int16-of-int32 read,
    and the pure-bf16 mask multiply engages the DVE 2x_1p mode.
  - ev/coef and x^T are precomputed on host; epilogue runs in bf16.
"""

import sys

for _p in ("/opt/trn_rl_repo", "/root/.axon_site/_ro/trn_rl_repo"):
    if _p not in sys.path:
        sys.path.append(_p)

import numpy as np
import ml_dtypes

import concourse.bass as bass
import concourse.mybir as mybir
from concourse import bacc
from concourse.tile import TileContext
from concourse.masks import make_identity
from concourse.bass_utils import run_bass_kernel_spmd

F32 = mybir.dt.float32
BF16 = mybir.dt.bfloat16

B, N, F, E = 8, 4096, 128, 128
RC = 512               # r-chunk width (one PSUM bank of fp32)
NB = N // 128          # 32 m-blocks
NRC = N // RC          # 8 r-chunks
T = RC // 128          # 4 sub-blocks per r-chunk
GJ = 2                 # j-tiles per psA/exp group (ACT instr = GJ*512 wide)
AJ = 4                 # j-tiles per adjacency DMA transfer
EXP_BIAS = -80.0

NPBF16 = ml_dtypes.bfloat16

_CACHED = {}


def _build():
    nc = bacc.Bacc("TRN2", target_bir_lowering=False, debug=False, num_devices=B)
    xb_d = nc.dram_tensor("xb", [128, NB, F], BF16, kind="ExternalInput").ap()
    xtb_d = nc.dram_tensor("xtb", [128, NB, 128], BF16, kind="ExternalInput").ap()
    adjt_d = nc.dram_tensor("adjt", [N, N], BF16, kind="ExternalInput").ap()
    evr_d = nc.dram_tensor("evr", [128, NB], F32, kind="ExternalInput").ap()
    cfr_d = nc.dram_tensor("cfr", [128, NB], F32, kind="ExternalInput").ap()
    w_d = nc.dram_tensor("w", [E, 2 * F], F32, kind="ExternalInput").ap()
    out_d = nc.dram_tensor("out", [N, E], F32, kind="ExternalOutput").ap()

    adjt_v = adjt_d.rearrange("(o p) c -> p o c", p=128)    # [128, NB, N]
    w_v = w_d.rearrange("e (h f) -> e h f", h=2)            # [128, 2, F]
    out_v = out_d.rearrange("(o p) e -> p o e", p=128)      # [128, NB, E]

    with TileContext(nc) as tc:
        with (
            tc.tile_pool(name="singles", bufs=1) as singles,
            tc.tile_pool(name="adj", bufs=4) as adj_pool,
            tc.tile_pool(name="et", bufs=3) as e_pool,
            tc.tile_pool(name="pt", bufs=6) as p_pool,
            tc.tile_pool(name="small", bufs=12) as small,
            tc.tile_pool(name="outp", bufs=6) as out_pool,
            tc.tile_pool(name="psumA", bufs=2, space="PSUM") as psum_a,
            tc.tile_pool(name="psumC", bufs=1, space="PSUM") as psum_c,
            tc.tile_pool(name="psumT", bufs=2, space="PSUM") as psum_t,
        ):
            # ---------------- setup ----------------
            # adjacency prefetch for rc=0 goes out first so the mask pipeline
            # never waits on it; remaining setup DMAs are chunked across
            # otherwise-idle queues so the first att matmul starts early.
            adj_prefetch = []
            for gpre in range(2):
                t_ = adj_pool.tile([128, AJ, RC], BF16, name="adj_tile")
                nc.gpsimd.dma_start(
                    out=t_[:], in_=adjt_v[:, gpre * AJ:(gpre + 1) * AJ, 0:RC])
                adj_prefetch.append(t_)

            # x^T bf16 [f part, m free] per block (host-transposed); needed
            # by the very first att matmul — split across two queues.
            xt_sb = singles.tile([128, NB, 128], BF16)
            nc.sync.dma_start(out=xt_sb[:, :NB // 2, :],
                              in_=xtb_d[:, :NB // 2, :])
            nc.scalar.dma_start(out=xt_sb[:, NB // 2:, :],
                                in_=xtb_d[:, NB // 2:, :])

            # moving operand of the aggregation matmul: bf16 x + ones column
            xb_sb = singles.tile([128, NB, F + 4], BF16)
            nc.gpsimd.dma_start(out=xb_sb[:, :NB // 2, :F],
                                in_=xb_d[:, :NB // 2, :])
            nc.scalar.dma_start(out=xb_sb[:, NB // 2:, :F],
                                in_=xb_d[:, NB // 2:, :])
            nc.vector.memset(xb_sb[:, :, F:F + 1], 1.0)

            expb = singles.tile([128, 1], F32)
            nc.vector.memset(expb[:], EXP_BIAS)

            ident = singles.tile([128, 128], F32)
            make_identity(nc, ident)
            ident_bf = singles.tile([128, 128], BF16)
            nc.vector.tensor_copy(ident_bf[:], ident[:])
            negbig_bf = singles.tile([128, 128], BF16)
            nc.vector.tensor_scalar_mul(negbig_bf[:], ident_bf[:], -30000.0)

            # W^T fp32 halves [f part, e free]
            w_sb = singles.tile([128, 2, F], F32)
            nc.sync.dma_start(out=w_sb[:], in_=w_v)
            wb_sb = singles.tile([128, 2, F], BF16)
            nc.vector.tensor_copy(wb_sb[:], w_sb[:])
            wt_sb = singles.tile([128, 2, E], BF16)
            for h in range(2):
                psb = psum_t.tile([128, 128], BF16, tag="tr")
                nc.tensor.transpose(psb[:], wb_sb[:, h, :], ident_bf[:])
                nc.vector.tensor_copy(wt_sb[:, h, :], psb[:])

            # analytic-diagonal coefficients, host-computed
            ev_sb = singles.tile([128, NB], F32)
            nc.sync.dma_start(out=ev_sb[:], in_=evr_d)
            coef_sb = singles.tile([128, NB], F32)
            nc.sync.dma_start(out=coef_sb[:], in_=cfr_d)

            # ---------------- main loop ----------------
            LAGP = 3              # quad lag in pair-groups
            pending = []          # (psumC_list, rc, g, pt_tile)

            # adjacency tile prefetch schedule: 8 tiles per rc, depth-2 ahead
            adj_sched = [(rc_, b_) for rc_ in range(NRC)
                         for b_ in range(0, NB, AJ)]
            adj_tiles = {0: adj_prefetch[0], 1: adj_prefetch[1]}

            def adj_ensure(i):
                if i >= len(adj_sched) or i in adj_tiles:
                    return
                rc_, b_ = adj_sched[i]
                t_ = adj_pool.tile([128, AJ, RC], BF16, name="adj_tile")
                dma = nc.gpsimd.dma_start if i % 2 == 0 else nc.sync.dma_start
                dma(out=t_[:],
                    in_=adjt_v[:, b_:b_ + AJ, rc_ * RC:(rc_ + 1) * RC])
                adj_tiles[i] = t_

            def emit_quads(item):
                psumC_l, rc_, g_, pt_ = item
                for jj in range(GJ):
                    j_ = g_ * GJ + jj
                    for t in range(T):
                        # Two accumulation chains share each PSUM bank. A
                        # start=True zeroes the whole 2KB bank, so only the
                        # first (slot-0) chain may issue it; the slot-1 chain
                        # inherits the bank's pending-zero for its range.
                        st = j_ == 0 and t % 2 == 0
                        nc.tensor.matmul(psumC_l[t // 2][:, t % 2, :],
                                         pt_[:, jj, t * 128:(t + 1) * 128],
                                         xb_sb[:, j_, 0:F + 1],
                                         start=st, stop=(j_ == NB - 1),
                                         skip_group_check=True)

            def epilogue_phase_a(psumC_l, rc_):
                # psumC -> SBUF copies; must all be emitted before the next
                # rc's first quads overwrite the accumulators.
                scs = []
                for t in range(T):
                    sc = small.tile([128, F + 1], BF16, name=f"sc{t}",
                                    tag=f"sc{t}")
                    nc.vector.tensor_copy(sc[:], psumC_l[t // 2][:, t % 2, :])
                    scs.append(sc)
                return scs

            def epilogue_phase_b(scs, rc_, t):
                    blk = rc_ * T + t
                    evb = ev_sb[:, blk:blk + 1]
                    cfb = coef_sb[:, blk:blk + 1]
                    sc = scs[t]
                    den = small.tile([128, 1], F32, tag="den")
                    nc.vector.scalar_tensor_tensor(
                        den[:], sc[:, F:F + 1], evb, cfb,
                        mybir.AluOpType.mult, mybir.AluOpType.add)
                    rden = small.tile([128, 1], F32, tag="rden")
                    nc.vector.reciprocal(rden[:], den[:])
                    xs = small.tile([128, F], BF16, tag="xs")
                    nc.vector.tensor_scalar_mul(xs[:], xb_sb[:, blk, :F], cfb)
                    cu = small.tile([128, F], BF16, tag="cu")
                    nc.vector.scalar_tensor_tensor(
                        cu[:], sc[:, 0:F], evb, xs[:],
                        mybir.AluOpType.mult, mybir.AluOpType.add)
                    cn = small.tile([128, F], BF16, tag="cn")
                    nc.vector.tensor_scalar_mul(cn[:], cu[:], rden[:])

                    psT = psum_t.tile([128, 128], BF16, tag="tr")
                    nc.tensor.transpose(psT[:], cn[:], ident_bf[:])
                    cnT = small.tile([128, F], BF16, tag="cnT")
                    nc.vector.tensor_copy(cnT[:], psT[:])

                    psF = psum_t.tile([128, E], F32, tag="tr")
                    nc.tensor.matmul(psF[:], xt_sb[:, blk, :], wt_sb[:, 0, :],
                                     start=True, stop=False)
                    nc.tensor.matmul(psF[:], cnT[:], wt_sb[:, 1, :],
                                     start=False, stop=True)
                    ot = out_pool.tile([128, E], F32)
                    nc.vector.tensor_relu(ot[:], psF[:])
                    nc.sync.dma_start(out=out_v[:, blk, :], in_=ot[:])

            NG = NB // GJ
            adj_tile = None
            epi_b = []            # deferred per-t epilogue work of prev rc
            for rc in range(NRC):
                psumC = [psum_c.tile([128, 2, F + 1], F32, name=f"psumC{h}",
                                     tag=f"psumC{h}") for h in range(T // 2)]
                for g in range(NB // GJ):
                    j0 = g * GJ
                    if j0 % AJ == 0:
                        i_adj = rc * (NB // AJ) + j0 // AJ
                        adj_ensure(i_adj)
                        adj_ensure(i_adj + 1)
                        adj_ensure(i_adj + 2)
                        adj_tile = adj_tiles.pop(i_adj)

                    psA = psum_a.tile([128, GJ, RC], F32)
                    for jj in range(GJ):
                        j = j0 + jj
                        diag = rc * T <= j < (rc + 1) * T
                        nc.tensor.matmul(psA[:, jj, :], xt_sb[:, j, :],
                                         xt_sb[:, rc * T:(rc + 1) * T, :],
                                         start=True, stop=not diag,
                                         skip_group_check=True)
                        if diag:
                            off = (j - rc * T) * 128
                            nc.tensor.matmul(psA[:, jj, off:off + 128],
                                             negbig_bf[:], ident_bf[:],
                                             start=False, stop=True,
                                             skip_group_check=True)

                    et = e_pool.tile([128, GJ, RC], BF16)
                    nc.scalar.activation(et[:], psA[:],
                                         mybir.ActivationFunctionType.Exp,
                                         bias=expb[:])
                    pt = p_pool.tile([128, GJ, RC], BF16)
                    nc.vector.tensor_tensor(
                        pt[:], et[:], adj_tile[:, j0 % AJ:j0 % AJ + GJ, :],
                        mybir.AluOpType.mult)

                    pending.append((psumC, rc, g, pt))
                    # eager drain toward rc end so the boundary has no
                    # quad backlog stalling the next rc's att matmuls
                    npop = 0
                    if len(pending) > LAGP:
                        npop = 1
                    if g >= NG - 4:
                        npop = min(2, len(pending))
                    if g == NG - 1:
                        npop = len(pending)
                    for _ in range(npop):
                        emit_quads(pending.pop(0))
                    # spread previous rc's epilogue across early groups
                    if epi_b and g >= 1:
                        scs_, rc_, t_ = epi_b.pop(0)
                        epilogue_phase_b(scs_, rc_, t_)

                scs = epilogue_phase_a(psumC, rc)
                epi_b.extend((scs, rc, t) for t in range(T))

            for scs_, rc_, t_ in epi_b:
                epilogue_phase_b(scs_, rc_, t_)

    nc.compile()
    return nc


def _get_nc():
    if "nc" not in _CACHED:
        _CACHED["nc"] = _build()
    return _CACHED["nc"]


def kernel(**inputs) -> np.ndarray:
    x_all = np.asarray(inputs["node_features"], dtype=np.float32)   # [B, N, F]
    adj_all = np.asarray(inputs["adj_list"])                        # [B, N, N] int32
    W = np.asarray(inputs["W"], dtype=np.float32)                   # [E, 2F]

    nc = _get_nc()
    in_maps = []
    for b in range(B):
        xr = x_all[b].reshape(NB, 128, F)
        xb = np.ascontiguousarray(xr.transpose(1, 0, 2)).astype(NPBF16)
        xtb = np.ascontiguousarray(xr.transpose(2, 0, 1)).astype(NPBF16)
        # adjt[m, r] = adj[r, m]; 0/1 -> bf16 via bit trick (0x3F80 = 1.0)
        adjt = np.ascontiguousarray(adj_all[b].T)
        adjt = (adjt.astype(np.uint16) * np.uint16(0x3F80)).view(NPBF16)
        d = (x_all[b] * x_all[b]).sum(-1).astype(np.float32)
        adiag = np.diagonal(adj_all[b]).astype(np.float32)
        ev = np.exp(-adiag * np.maximum(d - 110.0, 0.0)).astype(np.float32)
        cf = (adiag * np.exp(np.minimum(d - 80.0, 30.0))).astype(np.float32)
        evr = np.ascontiguousarray(ev.reshape(NB, 128).T)
        cfr = np.ascontiguousarray(cf.reshape(NB, 128).T)
        in_maps.append({
            "xb": xb,
            "xtb": xtb,
            "adjt": adjt,
            "evr": evr,
            "cfr": cfr,
            "w": W,
        })

    res = run_bass_kernel_spmd(nc, in_maps, core_ids=list(range(B)))
    out = np.stack([res.results[b]["out"] for b in range(B)], axis=0)
    return out.astype(np.float32, copy=False)


# revision 29
# speedup vs baseline: 1.0238x; 1.0238x over previous
"""Trainium2 Bass kernel for nn_Attention_aggregator (B=8, N=4096, F=128, E=128).

Sharding: data-parallel over batch — one batch element per NeuronCore (8 cores).
Each core computes, for its batch b:
    att  = x @ x.T                        [N, N]   (symmetric)
    att  = where(adj==0, -9999999, att)
    sm   = softmax(att, axis=-1)
    comb = sm @ x                         [N, F]
    out  = relu(concat([x, comb], -1) @ W.T)      [N, E]

Device decomposition (transposed orientation so the aggregation matmul's
contraction dim lands on partitions; attention symmetry makes the transposed
logits free):
    E^T[m, r] = exp(att[m, r] - 80)       (att[m,r] == att[r,m])
    diagonal of att killed in PSUM by an accumulating (-30000*I) @ I matmul
    P^T = E^T * adjT                      (adjT prepacked to bf16 on host)
    [S2 | S1] = P^T.T @ [x | 1]           (ones column => row-sum in column F)
    comb = (ev*S2 + coef*x) / (ev*S1 + coef)   with d = ||x_r||^2,
        ev = exp(-adj_rr*max(0, d-110)),  coef = adj_rr*exp(min(d-80, 30))
    (diagonal handled analytically: its logit is the only one that can
     overflow exp; everything off-diagonal is bounded ~|att|<70)
    out = relu([x, comb] @ W.T) with comb'^T stationary matmuls.

Perf structure vs the previous version:
  - exp is issued as one ACTIVATE per TWO 512-wide tiles ([128,1024] PSUM
    region, 2 banks) — halves the ~300ns fixed ACT overhead per tile.
  - adjacency is host-converted to bf16 (values 0/1 exact) and streamed
    contiguously — 2x DMA efficiency vs the strided int16-of-int32 read,
    and the pure-bf16 mask multiply engages the DVE 2x_1p mode.
  - ev/coef and x^T are precomputed on host; epilogue runs in bf16.
"""

import sys

for _p in ("/opt/trn_rl_repo", "/root/.axon_site/_ro/trn_rl_repo"):
    if _p not in sys.path:
        sys.path.append(_p)

import numpy as np
import ml_dtypes

import concourse.bass as bass
import concourse.mybir as mybir
from concourse import bacc
from concourse.tile import TileContext
from concourse.masks import make_identity
from concourse.bass_utils import run_bass_kernel_spmd

F32 = mybir.dt.float32
BF16 = mybir.dt.bfloat16

B, N, F, E = 8, 4096, 128, 128
RC = 512               # r-chunk width (one PSUM bank of fp32)
NB = N // 128          # 32 m-blocks
NRC = N // RC          # 8 r-chunks
T = RC // 128          # 4 sub-blocks per r-chunk
GJ = 2                 # j-tiles per psA/exp group (ACT instr = GJ*512 wide)
AJ = 4                 # j-tiles per adjacency DMA transfer
EXP_BIAS = -80.0

NPBF16 = ml_dtypes.bfloat16

_CACHED = {}


def _build():
    nc = bacc.Bacc("TRN2", target_bir_lowering=False, debug=False, num_devices=B)
    xb_d = nc.dram_tensor("xb", [128, NB, F], BF16, kind="ExternalInput").ap()
    xtb_d = nc.dram_tensor("xtb", [128, NB, 128], BF16, kind="ExternalInput").ap()
    adjt_d = nc.dram_tensor("adjt", [N, N], BF16, kind="ExternalInput").ap()
    evr_d = nc.dram_tensor("evr", [128, NB], F32, kind="ExternalInput").ap()
    cfr_d = nc.dram_tensor("cfr", [128, NB], F32, kind="ExternalInput").ap()
    w_d = nc.dram_tensor("w", [E, 2 * F], F32, kind="ExternalInput").ap()
    out_d = nc.dram_tensor("out", [N, E], F32, kind="ExternalOutput").ap()

    adjt_v = adjt_d.rearrange("(o p) c -> p o c", p=128)    # [128, NB, N]
    w_v = w_d.rearrange("e (h f) -> e h f", h=2)            # [128, 2, F]
    out_v = out_d.rearrange("(o p) e -> p o e", p=128)      # [128, NB, E]

    with TileContext(nc) as tc:
        with (
            tc.tile_pool(name="singles", bufs=1) as singles,
            tc.tile_pool(name="adj", bufs=5) as adj_pool,
            tc.tile_pool(name="et", bufs=4) as e_pool,
            tc.tile_pool(name="pt", bufs=6) as p_pool,
            tc.tile_pool(name="small", bufs=12) as small,
            tc.tile_pool(name="outp", bufs=6) as out_pool,
            tc.tile_pool(name="psumA", bufs=2, space="PSUM") as psum_a,
            tc.tile_pool(name="psumC", bufs=1, space="PSUM") as psum_c,
            tc.tile_pool(name="psumT", bufs=2, space="PSUM") as psum_t,
        ):
            # ---------------- setup ----------------
            # adjacency prefetch for rc=0 goes out first so the mask pipeline
            # never waits on it; remaining setup DMAs are chunked across
            # otherwise-idle queues so the first att matmul starts early.
            adj_prefetch = []
            for gpre in range(2):
                t_ = adj_pool.tile([128, AJ, RC], BF16, name="adj_tile")
                nc.gpsimd.dma_start(
                    out=t_[:], in_=adjt_v[:, gpre * AJ:(gpre + 1) * AJ, 0:RC])
                adj_prefetch.append(t_)

            # x^T bf16 [f part, m free] per block (host-transposed); needed
            # by the very first att matmul — split across two queues.
            xt_sb = singles.tile([128, NB, 128], BF16)
            nc.sync.dma_start(out=xt_sb[:, :NB // 2, :],
                              in_=xtb_d[:, :NB // 2, :])
            nc.scalar.dma_start(out=xt_sb[:, NB // 2:, :],
                                in_=xtb_d[:, NB // 2:, :])

            # moving operand of the aggregation matmul: bf16 x + ones column
            xb_sb = singles.tile([128, NB, F + 4], BF16)
            nc.gpsimd.dma_start(out=xb_sb[:, :NB // 2, :F],
                                in_=xb_d[:, :NB // 2, :])
            nc.scalar.dma_start(out=xb_sb[:, NB // 2:, :F],
                                in_=xb_d[:, NB // 2:, :])
            nc.vector.memset(xb_sb[:, :, F:F + 1], 1.0)

            expb = singles.tile([128, 1], F32)
            nc.vector.memset(expb[:], EXP_BIAS)

            ident = singles.tile([128, 128], F32)
            make_identity(nc, ident)
            ident_bf = singles.tile([128, 128], BF16)
            nc.vector.tensor_copy(ident_bf[:], ident[:])
            negbig_bf = singles.tile([128, 128], BF16)
            nc.vector.tensor_scalar_mul(negbig_bf[:], ident_bf[:], -30000.0)

            # W^T fp32 halves [f part, e free]
            w_sb = singles.tile([128, 2, F], F32)
            nc.sync.dma_start(out=w_sb[:], in_=w_v)
            wb_sb = singles.tile([128, 2, F], BF16)
            nc.vector.tensor_copy(wb_sb[:], w_sb[:])
            wt_sb = singles.tile([128, 2, E], BF16)
            for h in range(2):
                psb = psum_t.tile([128, 128], BF16, tag="tr")
                nc.tensor.transpose(psb[:], wb_sb[:, h, :], ident_bf[:])
                nc.vector.tensor_copy(wt_sb[:, h, :], psb[:])

            # analytic-diagonal coefficients, host-computed
            ev_sb = singles.tile([128, NB], F32)
            nc.sync.dma_start(out=ev_sb[:], in_=evr_d)
            coef_sb = singles.tile([128, NB], F32)
            nc.sync.dma_start(out=coef_sb[:], in_=cfr_d)

            # ---------------- main loop ----------------
            LAGP = 3              # quad lag in pair-groups
            pending = []          # (psumC_list, rc, g, pt_tile)

            # adjacency tile prefetch schedule: 8 tiles per rc, depth-2 ahead
            adj_sched = [(rc_, b_) for rc_ in range(NRC)
                         for b_ in range(0, NB, AJ)]
            adj_tiles = {0: adj_prefetch[0], 1: adj_prefetch[1]}

            def adj_ensure(i):
                if i >= len(adj_sched) or i in adj_tiles:
                    return
                rc_, b_ = adj_sched[i]
                t_ = adj_pool.tile([128, AJ, RC], BF16, name="adj_tile")
                dma = nc.gpsimd.dma_start if i % 2 == 0 else nc.sync.dma_start
                dma(out=t_[:],
                    in_=adjt_v[:, b_:b_ + AJ, rc_ * RC:(rc_ + 1) * RC])
                adj_tiles[i] = t_

            def emit_quads(item):
                psumC_l, rc_, g_, pt_ = item
                for jj in range(GJ):
                    j_ = g_ * GJ + jj
                    for t in range(T):
                        # Two accumulation chains share each PSUM bank. A
                        # start=True zeroes the whole 2KB bank, so only the
                        # first (slot-0) chain may issue it; the slot-1 chain
                        # inherits the bank's pending-zero for its range.
                        st = j_ == 0 and t % 2 == 0
                        nc.tensor.matmul(psumC_l[t // 2][:, t % 2, :],
                                         pt_[:, jj, t * 128:(t + 1) * 128],
                                         xb_sb[:, j_, 0:F + 1],
                                         start=st, stop=(j_ == NB - 1),
                                         skip_group_check=True)

            def epilogue_phase_a(psumC_l, rc_):
                # psumC -> SBUF copies; must all be emitted before the next
                # rc's first quads overwrite the accumulators.
                scs = []
                for t in range(T):
                    sc = small.tile([128, F + 1], BF16, name=f"sc{t}",
                                    tag=f"sc{t}")
                    nc.vector.tensor_copy(sc[:], psumC_l[t // 2][:, t % 2, :])
                    scs.append(sc)
                return scs

            def epilogue_phase_b(scs, rc_, t):
                    blk = rc_ * T + t
                    evb = ev_sb[:, blk:blk + 1]
                    cfb = coef_sb[:, blk:blk + 1]
                    sc = scs[t]
                    den = small.tile([128, 1], F32, tag="den")
                    nc.vector.scalar_tensor_tensor(
                        den[:], sc[:, F:F + 1], evb, cfb,
                        mybir.AluOpType.mult, mybir.AluOpType.add)
                    rden = small.tile([128, 1], F32, tag="rden")
                    nc.vector.reciprocal(rden[:], den[:])
                    xs = small.tile([128, F], BF16, tag="xs")
                    nc.vector.tensor_scalar_mul(xs[:], xb_sb[:, blk, :F], cfb)
                    cu = small.tile([128, F], BF16, tag="cu")
                    nc.vector.scalar_tensor_tensor(
                        cu[:], sc[:, 0:F], evb, xs[:],
                        mybir.AluOpType.mult, mybir.AluOpType.add)
                    cn = small.tile([128, F], BF16, tag="cn")
                    nc.vector.tensor_scalar_mul(cn[:], cu[:], rden[:])

                    psT = psum_t.tile([128, 128], BF16, tag="tr")
                    nc.tensor.transpose(psT[:], cn[:], ident_bf[:])
                    cnT = small.tile([128, F], BF16, tag="cnT")
                    nc.vector.tensor_copy(cnT[:], psT[:])

                    psF = psum_t.tile([128, E], F32, tag="tr")
                    nc.tensor.matmul(psF[:], xt_sb[:, blk, :], wt_sb[:, 0, :],
                                     start=True, stop=False)
                    nc.tensor.matmul(psF[:], cnT[:], wt_sb[:, 1, :],
                                     start=False, stop=True)
                    ot = out_pool.tile([128, E], F32)
                    nc.vector.tensor_relu(ot[:], psF[:])
                    nc.sync.dma_start(out=out_v[:, blk, :], in_=ot[:])

            NG = NB // GJ
            adj_tile = None
            epi_b = []            # deferred per-t epilogue work of prev rc
            for rc in range(NRC):
                psumC = [psum_c.tile([128, 2, F + 1], F32, name=f"psumC{h}",
                                     tag=f"psumC{h}") for h in range(T // 2)]
                for g in range(NB // GJ):
                    j0 = g * GJ
                    if j0 % AJ == 0:
                        i_adj = rc * (NB // AJ) + j0 // AJ
                        adj_ensure(i_adj)
                        adj_ensure(i_adj + 1)
                        adj_ensure(i_adj + 2)
                        adj_tile = adj_tiles.pop(i_adj)

                    psA = psum_a.tile([128, GJ, RC], F32)
                    for jj in range(GJ):
                        j = j0 + jj
                        diag = rc * T <= j < (rc + 1) * T
                        nc.tensor.matmul(psA[:, jj, :], xt_sb[:, j, :],
                                         xt_sb[:, rc * T:(rc + 1) * T, :],
                                         start=True, stop=not diag,
                                         skip_group_check=True)
                        if diag:
                            off = (j - rc * T) * 128
                            nc.tensor.matmul(psA[:, jj, off:off + 128],
                                             negbig_bf[:], ident_bf[:],
                                             start=False, stop=True,
                                             skip_group_check=True)

                    et = e_pool.tile([128, GJ, RC], BF16)
                    nc.scalar.activation(et[:], psA[:],
                                         mybir.ActivationFunctionType.Exp,
                                         bias=expb[:])
                    pt = p_pool.tile([128, GJ, RC], BF16)
                    nc.vector.tensor_tensor(
                        pt[:], et[:], adj_tile[:, j0 % AJ:j0 % AJ + GJ, :],
                        mybir.AluOpType.mult)

                    pending.append((psumC, rc, g, pt))
                    # eager drain toward rc end so the boundary has no
                    # quad backlog stalling the next rc's att matmuls
                    npop = 0
                    if len(pending) > LAGP:
                        npop = 1
                    if g >= NG - 4:
                        npop = min(2, len(pending))
                    if g == NG - 1:
                        npop = len(pending)
                    for _ in range(npop):
                        emit_quads(pending.pop(0))
                    # spread previous rc's epilogue across early groups
                    if epi_b and g >= 1:
                        scs_, rc_, t_ = epi_b.pop(0)
                        epilogue_phase_b(scs_, rc_, t_)

                scs = epilogue_phase_a(psumC, rc)
                epi_b.extend((scs, rc, t) for t in range(T))

            for scs_, rc_, t_ in epi_b:
                epilogue_phase_b(scs_, rc_, t_)

    nc.compile()
    return nc


def _get_nc():
    if "nc" not in _CACHED:
        _CACHED["nc"] = _build()
    return _CACHED["nc"]


def kernel(**inputs) -> np.ndarray:
    x_all = np.asarray(inputs["node_features"], dtype=np.float32)   # [B, N, F]
    adj_all = np.asarray(inputs["adj_list"])                        # [B, N, N] int32
    W = np.asarray(inputs["W"], dtype=np.float32)                   # [E, 2F]

    nc = _get_nc()
    in_maps = []
    for b in range(B):
        xr = x_all[b].reshape(NB, 128, F)
        xb = np.ascontiguousarray(xr.transpose(1, 0, 2)).astype(NPBF16)
        xtb = np.ascontiguousarray(xr.transpose(2, 0, 1)).astype(NPBF16)
        # adjt[m, r] = adj[r, m]; 0/1 -> bf16 via bit trick (0x3F80 = 1.0)
        adjt = np.ascontiguousarray(adj_all[b].T)
        adjt = (adjt.astype(np.uint16) * np.uint16(0x3F80)).view(NPBF16)
        d = (x_all[b] * x_all[b]).sum(-1).astype(np.float32)
        adiag = np.diagonal(adj_all[b]).astype(np.float32)
        ev = np.exp(-adiag * np.maximum(d - 110.0, 0.0)).astype(np.float32)
        cf = (adiag * np.exp(np.minimum(d - 80.0, 30.0))).astype(np.float32)
        evr = np.ascontiguousarray(ev.reshape(NB, 128).T)
        cfr = np.ascontiguousarray(cf.reshape(NB, 128).T)
        in_maps.append({
            "xb": xb,
            "xtb": xtb,
            "adjt": adjt,
            "evr": evr,
            "cfr": cfr,
            "w": W,
        })

    res = run_bass_kernel_spmd(nc, in_maps, core_ids=list(range(B)))
    out = np.stack([res.results[b]["out"] for b in range(B)], axis=0)
    return out.astype(np.float32, copy=False)
